# revision 21
# baseline (speedup 1.0000x reference)
"""Trainium2 Bass kernel for nn_AttentionRNN (embedding + masked GRU + MLP head + softmax).

Strategy (pure data parallelism over 8 NeuronCores, 2048 examples/core):

The GRU runs transposed — state h kept as hT [H=128 partitions, examples on
free dim], so the recurrence is closed under the layout.

Embedding lookup: 4 transpose-mode dma_gather calls per step (512 idxs each,
round-robin across 4 SWDGE queues so descriptor generation runs on 4 Q7 core
pairs in parallel, ~5us/step total) fetch 256B rows of xtab [V, 128] f16:
  xtab[v] = [emb[v] (32) | mask_ind (BIGM if v==0 else 0) | 0-pad (95)]
directly into feature-on-partition layout xaT [128, 2048] (partition f =
augmented-input feature f, token on free dim) — no on-chip transposes needed.
The mask indicator row of the augmented weight matrix wa adds BIGM to the
z-gate preactivation of masked steps => z = sigmoid(>=BIGM - |U@h|) == 1
=> h' = h exactly.

Each gathered 512B row = [x-aug block (256B) | pre-projected h-gate block
(256B)]: block 0 = [emb(32) | mask_ind | const 1 | pad] feeding the z/r
matmuls (K=34); block 1 = emb@W_h + b0_h (the gtab trick), consumed directly
by the DVE t2-add — no h-gate input matmul at all.

Per step t, per 512-example group (psum [128, 1024] = z|r + 1 bank pG):

  z-pre = U_z.T @ hT + Wa_z.T @ xaT   (Wa_z row 32 = 1s -> +BIGM when masked)
  r-pre = U_r.T @ hT + Wa_r.T @ xaT
  pG    = U_h.T @ hT                              (recurrent h-gate part)
  z|r  = sigmoid([z-pre | r-pre])                 (one ACT call, reads PSUM)
  t1   = (pG + b1_h) * r                          (DVE scalar_tensor_tensor)
  t2   = t1 + xghT                                (DVE add, gathered block 1)
  hh   = tanh(t2)                                 (ACT)
  h'   = z*(h - hh) + hh                          (3 DVE tensor_tensor ops)

Biases (zero for the graded input): b1_h via the STT scalar, b0_h baked into
the gathered block, b0_z|r + b1_z|r via K=1 matmuls (czr) only when nonzero.

Head: dT = swish(W1.T @ hT + b1); logits per 128-example tile with examples on
partitions (lhsT = dT slice); softmax along free dim (C=3).
"""

import numpy as np
from contextlib import ExitStack

import concourse.mybir as mybir
import concourse.tile as tile
from concourse import bacc
from concourse.bass_utils import run_bass_kernel_spmd

B, T, E, H, V, D, C = 16384, 128, 32, 128, 30001, 128, 3
NCORES = 8
BC = B // NCORES
BIGM = 100.0
RW = 256               # xtab row pitch in f16: [x-aug block | pre-projected h-gate block]
XW = 34                # useful columns of xtab (= rows of wa): E emb + mask + const
NQ = 4                 # SWDGE queues; gather calls round-robin across them
PERCALL = 512          # idxs per dma_gather call
DMA_SCRATCH = 49152    # SWDGE descriptor-ring carveout (bytes/partition)
QUEUE_PLAN = [0, 1, 2, 3]  # queue for call c
G_BUFS = 6
H_BUFS = 3
Z_BUFS = 3
TMP_BUFS = 2
XA_BUFS = 2
PW = 512               # width of the blend DVE ops
PZR_BUFS = 2           # psum pool depths (8 banks: 2*2 PZR + 2 PG + 2 PX)
NFAT = 3               # calls gathering the pre-projected h-gate block
PG_BUFS = 2
PX_BUFS = 2
COPY_ENGINE = "act"    # psum->sbuf xaT copies: "act" or "dve"

F16 = mybir.dt.float16
F32 = mybir.dt.float32
I16 = mybir.dt.int16
AF = mybir.ActivationFunctionType
OP = mybir.AluOpType


def build_nc(bc=BC, nt=T, with_czr=False):
    """Build + compile the per-core Bass program. bc = examples per core."""
    assert bc % 512 == 0
    ng = bc // 512            # 512-example groups per step
    kb = bc // 128            # gathered rows per partition per step
    pw = min(PW, bc)
    npairs = bc // pw
    gperp = pw // 512

    nc = bacc.Bacc("TRN2", target_bir_lowering=False, debug=False,
                   num_swdge_queues=NQ, dynamic_dma_scratch_size=DMA_SCRATCH)
    xtab = nc.dram_tensor("xtab", [V, RW], F16, kind="ExternalInput").ap()
    xtab2 = nc.dram_tensor("xtab2", [V, RW // 2], F16, kind="ExternalInput").ap()
    idxw = nc.dram_tensor("idxw", [128, nt * bc // 16], I16, kind="ExternalInput").ap()
    ua = nc.dram_tensor("ua", [128, 384], F16, kind="ExternalInput").ap()
    wa = nc.dram_tensor("wa", [XW, 256], F16, kind="ExternalInput").ap()
    wah = nc.dram_tensor("wah", [XW, 128], F16, kind="ExternalInput").ap()
    b1h = nc.dram_tensor("b1h", [128, 1], F32, kind="ExternalInput").ap()
    w1 = nc.dram_tensor("w1", [128, 128], F16, kind="ExternalInput").ap()
    b1c = nc.dram_tensor("b1c", [128, 1], F32, kind="ExternalInput").ap()
    wout = nc.dram_tensor("wout", [128, C], F16, kind="ExternalInput").ap()
    boutw = nc.dram_tensor("boutw", [1, C], F16, kind="ExternalInput").ap()
    if with_czr:
        czr = nc.dram_tensor("czr", [1, 256], F16, kind="ExternalInput").ap()
    outp = nc.dram_tensor("outp", [128, (bc // 128) * C], F32, kind="ExternalOutput").ap()

    with tile.TileContext(nc) as tc, ExitStack() as ctx:
        wp = ctx.enter_context(tc.tile_pool(name="w", bufs=1))
        ip = ctx.enter_context(tc.tile_pool(name="idx", bufs=1))
        gp = ctx.enter_context(tc.tile_pool(name="g", bufs=G_BUFS))
        hp = ctx.enter_context(tc.tile_pool(name="h", bufs=H_BUFS))
        zp = ctx.enter_context(tc.tile_pool(name="zr", bufs=Z_BUFS))
        tp = ctx.enter_context(tc.tile_pool(name="tmp", bufs=TMP_BUFS))
        pzr = ctx.enter_context(tc.tile_pool(name="pszr", bufs=PZR_BUFS, space="PSUM"))
        pg = ctx.enter_context(tc.tile_pool(name="psg", bufs=PG_BUFS, space="PSUM"))
        px = ctx.enter_context(tc.tile_pool(name="psx", bufs=PX_BUFS, space="PSUM"))
        hd = ctx.enter_context(tc.tile_pool(name="hd", bufs=2))

        ua_sb = wp.tile([128, 384], F16, tag="ua")
        nc.sync.dma_start(ua_sb[:], ua)
        wa_sb = wp.tile([XW, 256], F16, tag="wa")
        nc.sync.dma_start(wa_sb[:], wa)
        wah_sb = wp.tile([XW, 128], F16, tag="wah")
        nc.sync.dma_start(wah_sb[:], wah)
        b1h_sb = wp.tile([128, 1], F32, tag="b1h")
        nc.sync.dma_start(b1h_sb[:], b1h)
        w1_sb = wp.tile([128, 128], F16, tag="w1")
        nc.sync.dma_start(w1_sb[:], w1)
        b1c_sb = wp.tile([128, 1], F32, tag="b1c")
        nc.sync.dma_start(b1c_sb[:], b1c)
        wout_sb = wp.tile([128, C], F16, tag="wo")
        nc.sync.dma_start(wout_sb[:], wout)
        bout_sb = wp.tile([1, C], F16, tag="bo")
        nc.sync.dma_start(bout_sb[:], boutw)
        ones_sb = wp.tile([1, 128], F16, tag="ones")
        nc.vector.memset(ones_sb[:], 1.0)
        # Pin the ACT table set that contains BOTH Sigmoid and Tanh so the
        # auto-placement pass doesn't ping-pong table loads every step
        # (~1.3us per load on the ACT critical path).
        from concourse.hw_specs import get_activation_tables
        _tabs = get_activation_tables(nc.m.arch)
        _setid = next(i for i, (nm2, fs) in enumerate(_tabs.items())
                      if AF.Sigmoid in fs and AF.Tanh in fs)
        nc.scalar.add_instruction(mybir.InstLoadActFuncSet(
            name=nc.get_next_instruction_name(), ins=[], outs=[],
            act_func_set_id=_setid))
        if with_czr:
            czr_sb = wp.tile([1, 256], F16, tag="czr")
            nc.sync.dma_start(czr_sb[:], czr)
            onesbc_sb = wp.tile([1, bc], F16, tag="onesbc")
            nc.vector.memset(onesbc_sb[:], 1.0)
        idx_sb = ip.tile([128, nt * bc // 16], I16, tag="idx")
        nc.sync.dma_start(idx_sb[:], idxw)

        h = hp.tile([128, bc], F16, tag="h")
        nc.vector.memset(h[:], 0.0)

        for t in range(nt):
            ncall = bc // PERCALL
            xgs = []
            for c in range(ncall):
                xgc = gp.tile([128, 2, PERCALL], F16, tag="g")
                off = t * bc + c * PERCALL
                if c % ncall < NFAT:
                    nc.gpsimd.dma_gather(
                        xgc[:], xtab,
                        idx_sb[:, off // 16:(off + PERCALL) // 16],
                        PERCALL, PERCALL, RW, transpose=True,
                        queue_num=QUEUE_PLAN[c % len(QUEUE_PLAN)],
                    )
                else:
                    nc.gpsimd.dma_gather(
                        xgc[:, 0:1, :], xtab2,
                        idx_sb[:, off // 16:(off + PERCALL) // 16],
                        PERCALL, PERCALL, RW // 2, transpose=True,
                        queue_num=QUEUE_PLAN[c % len(QUEUE_PLAN)],
                    )
                xgs.append(xgc)

            zr = zp.tile([128, 2 * bc], F16, tag="zr")
            t1 = tp.tile([128, bc], F16, tag="t1")
            hnew = hp.tile([128, bc], F16, tag="h")
            t2 = tp.tile([128, bc], F16, tag="t2")
            hh = tp.tile([128, bc], F16, tag="hh")
            dd = tp.tile([128, bc], F16, tag="dd")
            m1 = tp.tile([128, bc], F16, tag="m1")
            for gi in range(ng):
                ps_t = pzr.tile([128, 1024], F32, tag="ps")
                ps = ps_t[:]
                pG_t = pg.tile([128, 512], F32, tag="pg")
                pG = pG_t[:]
                exs = slice(gi * 512, (gi + 1) * 512)
                nc.tensor.matmul(ps[:, 0:512], ua_sb[:, 0:128], h[:, exs], start=True, stop=False)
                nc.tensor.matmul(ps[:, 0:512], wa_sb[:, 0:128], xgs[gi][0:XW, 0, :],
                                 start=False, stop=not with_czr)
                if with_czr:
                    nc.tensor.matmul(ps[:, 0:512], czr_sb[:, 0:128], onesbc_sb[:, exs],
                                     start=False, stop=True)
                nc.tensor.matmul(ps[:, 512:1024], ua_sb[:, 128:256], h[:, exs], start=True, stop=False)
                nc.tensor.matmul(ps[:, 512:1024], wa_sb[:, 128:256], xgs[gi][0:XW, 0, :],
                                 start=False, stop=not with_czr)
                if with_czr:
                    nc.tensor.matmul(ps[:, 512:1024], czr_sb[:, 128:256], onesbc_sb[:, exs],
                                     start=False, stop=True)
                nc.tensor.matmul(pG, ua_sb[:, 256:384], h[:, exs], start=True, stop=True)
                fat = gi % ng < NFAT
                if not fat:
                    pG2_t = px.tile([128, 512], F32, tag="px")
                    pG2 = pG2_t[:]
                    nc.tensor.matmul(pG2, wah_sb[:], xgs[gi][0:XW, 0, :],
                                     start=True, stop=True)
                nc.scalar.activation(zr[:, gi * 1024:(gi + 1) * 1024], ps, AF.Sigmoid)
                nc.vector.scalar_tensor_tensor(
                    t1[:, exs], pG, b1h_sb[:],
                    zr[:, gi * 1024 + 512:(gi + 1) * 1024], OP.add, OP.mult,
                )
                if fat:
                    nc.vector.tensor_add(t2[:, exs], t1[:, exs], xgs[gi][:, 1, :])
                else:
                    nc.vector.tensor_add(t2[:, exs], t1[:, exs], pG2)

            def blend(pi):
                sl = slice(pi * pw, (pi + 1) * pw)
                nc.vector.tensor_sub(dd[:, sl], h[:, sl], hh[:, sl])
                zv = zr[:, pi * gperp * 1024:(pi + 1) * gperp * 1024] \
                    .rearrange("p (g c) -> p g c", g=gperp)[:, :, 0:512]
                dv = dd[:, sl].rearrange("p (g c) -> p g c", g=gperp)
                mv = m1[:, sl].rearrange("p (g c) -> p g c", g=gperp)
                nc.vector.tensor_mul(mv, zv, dv)
                nc.vector.tensor_add(hnew[:, sl], m1[:, sl], hh[:, sl])

            for pi in range(npairs):
                sl = slice(pi * pw, (pi + 1) * pw)
                nc.scalar.activation(hh[:, sl], t2[:, sl], AF.Tanh)
                blend(pi)
            h = hnew

        out_sb = hd.tile([128, (bc // 128) * C], F32, tag="out")
        et_all = hd.tile([128, (bc // 128) * C], F32, tag="eta")
        ss_all = hd.tile([128, (bc // 128)], F32, tag="ssa")
        for hg in range(bc // 512):
            psd_t = pzr.tile([128, 1024], F32, tag="ps")
            psd = psd_t[:, 0:512]
            nc.tensor.matmul(psd, w1_sb[:], h[:, hg * 512:(hg + 1) * 512], start=True, stop=True)
            sg = hd.tile([128, 512], F16, tag="sg")
            nc.scalar.activation(sg[:], psd, AF.Sigmoid, bias=b1c_sb[:])
            dt = hd.tile([128, 512], F16, tag="dt")
            # swish(d) = d * sigmoid(d), d = psd + b1
            nc.vector.scalar_tensor_tensor(dt[:], psd, b1c_sb[:], sg[:], OP.add, OP.mult)
            for sub in range(4):
                psl_t = pg.tile([128, 512], F32, tag="pg")
                psl = psl_t[:, 0:C]
                nc.tensor.matmul(psl, dt[:, sub * 128:(sub + 1) * 128], wout_sb[:], start=True, stop=False)
                nc.tensor.matmul(psl, ones_sb[:], bout_sb[:], start=False, stop=True)
                i = hg * 4 + sub
                nc.scalar.activation(et_all[:, i * C:(i + 1) * C], psl, AF.Exp,
                                     accum_out=ss_all[:, i:i + 1])
        rc_all = hd.tile([128, (bc // 128)], F32, tag="rc")
        nc.vector.reciprocal(rc_all[:], ss_all[:])
        for i in range(bc // 128):
            nc.vector.tensor_scalar_mul(out_sb[:, i * C:(i + 1) * C],
                                        et_all[:, i * C:(i + 1) * C], rc_all[:, i:i + 1])
        nc.sync.dma_start(outp, out_sb[:])

    nc.compile()
    return nc


def prep_tables(emb, W, U, b, W1, b1, Wout, bout):
    """Host-side weight preprocessing -> (shared input dict, with_czr flag)."""
    f16 = np.float16
    emb = np.asarray(emb, np.float64)
    W = np.asarray(W, np.float64)
    b = np.asarray(b, np.float64)
    xtab = np.zeros((V, RW), f16)
    xtab[:, 0:E] = emb.astype(f16)
    xtab[0, E] = np.float16(BIGM)        # mask indicator: token==0 freezes state
    xtab[:, E + 1] = 1.0                 # constant row (bias carrier for thin groups)
    # pre-projected h-gate input block (gtab trick, incl b0_h)
    xtab[:, H:2 * H] = (emb @ W[:, 2 * H:3 * H] + b[0, 2 * H:3 * H]).astype(f16)
    wa = np.zeros((XW, 2 * H), f16)
    wa[0:E, :] = W[:, 0:2 * H].astype(f16)
    wa[E, 0:H] = 1.0                     # routes the mask indicator into z-pre
    # constant z/r bias terms (b0+b1 both enter pre-sigmoid directly)
    c = b[0, 0:2 * H] + b[1, 0:2 * H]
    with_czr = bool(np.any(np.abs(c) > 1e-12))
    wah = np.zeros((XW, H), f16)
    wah[0:E, :] = W[:, 2 * H:3 * H].astype(f16)
    wah[E + 1, :] = b[0, 2 * H:3 * H].astype(f16)   # b0_h rides the const row
    shared = {
        "xtab": xtab,
        "xtab2": np.ascontiguousarray(xtab[:, 0:RW // 2]),
        "wah": wah,
        "ua": np.asarray(U, np.float32).astype(f16),
        "wa": wa,
        "b1h": np.asarray(b[1, 2 * H:3 * H], np.float32).reshape(128, 1).copy(),
        "w1": np.asarray(W1, np.float32).astype(f16),
        "b1c": np.asarray(b1, np.float32).reshape(128, 1).copy(),
        "wout": np.asarray(Wout, np.float32).astype(f16),
        "boutw": np.asarray(bout, np.float32).reshape(1, C).astype(f16),
    }
    if with_czr:
        shared["czr"] = c.reshape(1, 256).astype(f16)
    return shared, with_czr


def prep_idx(tokens_core, nt):
    """tokens_core [bc, nt] int -> idx tensor [128, nt*bc/16] int16.

    Per step t, per call c (PERCALL idxs), position i = e - c*PERCALL is
    wrapped: [16, PERCALL/16] column-major, replicated to 128 partitions.
    dma_gather(transpose=False) then writes row(e) to out[p=e%128, e//128]."""
    bc = tokens_core.shape[0]
    tk = np.asarray(tokens_core, np.int16)
    w = tk.T.reshape(nt * bc // 16, 16).transpose(1, 0)   # [16, nt*bc/16]
    return np.ascontiguousarray(np.tile(w, (8, 1)))


def assemble_out(res_core, bc=BC):
    """[128, (bc/128)*3] f32 device output -> [bc, 3] (example e = i*128 + p)."""
    return np.ascontiguousarray(
        res_core.reshape(128, bc // 128, C).transpose(1, 0, 2).reshape(bc, C)
    )


_NC_CACHE = {}


def kernel(tokens, emb, W, U, b, W1, b1, Wout, bout):
    tokens = np.asarray(tokens)
    shared, with_czr = prep_tables(emb, W, U, b, W1, b1, Wout, bout)
    key = (BC, T, with_czr)
    if key not in _NC_CACHE:
        _NC_CACHE[key] = build_nc(BC, T, with_czr)
    nc = _NC_CACHE[key]
    in_maps = []
    for c in range(NCORES):
        m = dict(shared)
        tc = tokens[c * BC:(c + 1) * BC]
        m["idxw"] = prep_idx(tc, T)
        in_maps.append(m)
    res = run_bass_kernel_spmd(nc, in_maps, core_ids=list(range(NCORES)))
    out = np.concatenate([assemble_out(res.results[c]["outp"], BC) for c in range(NCORES)], axis=0)
    return out.astype(np.float32)


# revision 22
# speedup vs baseline: 1.4378x; 1.4378x over previous
"""Trainium2 Bass kernel for nn_AttentionRNN (embedding + masked GRU + MLP head + softmax).

Strategy (pure data parallelism over 8 NeuronCores, 2048 examples/core):

The GRU runs transposed — state h kept as hT [H=128 partitions, examples on
free dim], so the recurrence is closed under the layout.

Embedding lookup: 4 transpose-mode dma_gather calls per step (512 idxs each,
round-robin across 4 SWDGE queues so descriptor generation runs on 4 Q7 core
pairs in parallel, ~5us/step total) fetch 256B rows of xtab [V, 128] f16:
  xtab[v] = [emb[v] (32) | mask_ind (BIGM if v==0 else 0) | 0-pad (95)]
directly into feature-on-partition layout xaT [128, 2048] (partition f =
augmented-input feature f, token on free dim) — no on-chip transposes needed.
The mask indicator row of the augmented weight matrix wa adds BIGM to the
z-gate preactivation of masked steps => z = sigmoid(>=BIGM - |U@h|) == 1
=> h' = h exactly.

Each gathered 512B row = [x-aug block (256B) | pre-projected h-gate block
(256B)]: block 0 = [emb(32) | mask_ind | const 1 | pad] feeding the z/r
matmuls (K=34); block 1 = emb@W_h + b0_h (the gtab trick), consumed directly
by the DVE t2-add — no h-gate input matmul at all.

Per step t, per 512-example group (psum [128, 1024] = z|r + 1 bank pG):

  z-pre = U_z.T @ hT + Wa_z.T @ xaT   (Wa_z row 32 = 1s -> +BIGM when masked)
  r-pre = U_r.T @ hT + Wa_r.T @ xaT
  pG    = U_h.T @ hT                              (recurrent h-gate part)
  z|r  = sigmoid([z-pre | r-pre])                 (one ACT call, reads PSUM)
  t1   = (pG + b1_h) * r                          (DVE scalar_tensor_tensor)
  t2   = t1 + xghT                                (DVE add, gathered block 1)
  hh   = tanh(t2)                                 (ACT)
  h'   = z*(h - hh) + hh                          (3 DVE tensor_tensor ops)

Biases (zero for the graded input): b1_h via the STT scalar, b0_h baked into
the gathered block, b0_z|r + b1_z|r via K=1 matmuls (czr) only when nonzero.

Head: dT = swish(W1.T @ hT + b1); logits per 128-example tile with examples on
partitions (lhsT = dT slice); softmax along free dim (C=3).
"""

import numpy as np
from contextlib import ExitStack

import concourse.mybir as mybir
import concourse.tile as tile
from concourse import bacc
from concourse.bass_utils import run_bass_kernel_spmd

B, T, E, H, V, D, C = 16384, 128, 32, 128, 30001, 128, 3
NCORES = 8
BC = B // NCORES
BIGM = 100.0
RW = 256               # xtab row pitch in f16: [x-aug block | pre-projected h-gate block]
XW = 34                # useful columns of xtab (= rows of wa): E emb + mask + const
NQ = 4                 # SWDGE queues; gather calls round-robin across them
PERCALL = 512          # idxs per dma_gather call
DMA_SCRATCH = 49152    # SWDGE descriptor-ring carveout (bytes/partition)
QUEUE_PLAN = [0, 1, 2, 3]  # queue for call c
G_BUFS = 6
H_BUFS = 3
Z_BUFS = 3
TMP_BUFS = 2
XA_BUFS = 2
PW = 512               # width of the blend DVE ops
PZR_BUFS = 3           # psum pool depths (8 banks: 3*PZR + PG)
PG_BUFS = 2
PX_BUFS = 2
COPY_ENGINE = "act"    # psum->sbuf xaT copies: "act" or "dve"

F16 = mybir.dt.float16
F32 = mybir.dt.float32
I16 = mybir.dt.int16
AF = mybir.ActivationFunctionType
OP = mybir.AluOpType


def build_nc(bc=BC, nt=T, with_czr=False):
    """Build + compile the per-core Bass program. bc = examples per core."""
    assert bc % 512 == 0
    ng = bc // 512            # 512-example groups per step
    kb = bc // 128            # gathered rows per partition per step
    pw = min(PW, bc)
    npairs = bc // pw
    gperp = pw // 512

    nc = bacc.Bacc("TRN2", target_bir_lowering=False, debug=False,
                   num_swdge_queues=NQ, dynamic_dma_scratch_size=DMA_SCRATCH)
    xtab = nc.dram_tensor("xtab", [V, RW], F16, kind="ExternalInput").ap()
    idxw = nc.dram_tensor("idxw", [128, nt * bc // 16], I16, kind="ExternalInput").ap()
    ua = nc.dram_tensor("ua", [128, 384], F16, kind="ExternalInput").ap()
    wa = nc.dram_tensor("wa", [XW, 256], F16, kind="ExternalInput").ap()
    b1h = nc.dram_tensor("b1h", [128, 1], F32, kind="ExternalInput").ap()
    w1 = nc.dram_tensor("w1", [128, 128], F16, kind="ExternalInput").ap()
    b1c = nc.dram_tensor("b1c", [128, 1], F32, kind="ExternalInput").ap()
    wout = nc.dram_tensor("wout", [128, C], F16, kind="ExternalInput").ap()
    boutw = nc.dram_tensor("boutw", [1, C], F16, kind="ExternalInput").ap()
    if with_czr:
        czr = nc.dram_tensor("czr", [1, 256], F16, kind="ExternalInput").ap()
    outp = nc.dram_tensor("outp", [128, (bc // 128) * C], F32, kind="ExternalOutput").ap()

    with tile.TileContext(nc) as tc, ExitStack() as ctx:
        wp = ctx.enter_context(tc.tile_pool(name="w", bufs=1))
        ip = ctx.enter_context(tc.tile_pool(name="idx", bufs=1))
        gp = ctx.enter_context(tc.tile_pool(name="g", bufs=G_BUFS))
        hp = ctx.enter_context(tc.tile_pool(name="h", bufs=H_BUFS))
        zp = ctx.enter_context(tc.tile_pool(name="zr", bufs=Z_BUFS))
        tp = ctx.enter_context(tc.tile_pool(name="tmp", bufs=TMP_BUFS))
        pzr = ctx.enter_context(tc.tile_pool(name="pszr", bufs=PZR_BUFS, space="PSUM"))
        pg = ctx.enter_context(tc.tile_pool(name="psg", bufs=PG_BUFS, space="PSUM"))
        hd = ctx.enter_context(tc.tile_pool(name="hd", bufs=2))

        ua_sb = wp.tile([128, 384], F16, tag="ua")
        nc.sync.dma_start(ua_sb[:], ua)
        wa_sb = wp.tile([XW, 256], F16, tag="wa")
        nc.sync.dma_start(wa_sb[:], wa)
        b1h_sb = wp.tile([128, 1], F32, tag="b1h")
        nc.sync.dma_start(b1h_sb[:], b1h)
        w1_sb = wp.tile([128, 128], F16, tag="w1")
        nc.sync.dma_start(w1_sb[:], w1)
        b1c_sb = wp.tile([128, 1], F32, tag="b1c")
        nc.sync.dma_start(b1c_sb[:], b1c)
        wout_sb = wp.tile([128, C], F16, tag="wo")
        nc.sync.dma_start(wout_sb[:], wout)
        bout_sb = wp.tile([1, C], F16, tag="bo")
        nc.sync.dma_start(bout_sb[:], boutw)
        ones_sb = wp.tile([1, 128], F16, tag="ones")
        nc.vector.memset(ones_sb[:], 1.0)
        # Pin the ACT table set that contains BOTH Sigmoid and Tanh so the
        # auto-placement pass doesn't ping-pong table loads every step
        # (~1.3us per load on the ACT critical path).
        from concourse.hw_specs import get_activation_tables
        _tabs = get_activation_tables(nc.m.arch)
        _setid = next(i for i, (nm2, fs) in enumerate(_tabs.items())
                      if AF.Sigmoid in fs and AF.Tanh in fs)
        nc.scalar.add_instruction(mybir.InstLoadActFuncSet(
            name=nc.get_next_instruction_name(), ins=[], outs=[],
            act_func_set_id=_setid))
        if with_czr:
            czr_sb = wp.tile([1, 256], F16, tag="czr")
            nc.sync.dma_start(czr_sb[:], czr)
            onesbc_sb = wp.tile([1, bc], F16, tag="onesbc")
            nc.vector.memset(onesbc_sb[:], 1.0)
        idx_sb = ip.tile([128, nt * bc // 16], I16, tag="idx")
        nc.sync.dma_start(idx_sb[:], idxw)

        h = hp.tile([128, bc], F16, tag="h")
        nc.vector.memset(h[:], 0.0)

        for t in range(nt):
            ncall = bc // PERCALL
            xgs = []
            for c in range(ncall):
                xgc = gp.tile([128, 2, PERCALL], F16, tag="g")
                off = t * bc + c * PERCALL
                nc.gpsimd.dma_gather(
                    xgc[:], xtab,
                    idx_sb[:, off // 16:(off + PERCALL) // 16],
                    PERCALL, PERCALL, RW, transpose=True,
                    queue_num=QUEUE_PLAN[c % len(QUEUE_PLAN)],
                )
                xgs.append(xgc)

            zr = zp.tile([128, 2 * bc], F16, tag="zr")
            t1 = tp.tile([128, bc], F16, tag="t1")
            hnew = hp.tile([128, bc], F16, tag="h")
            t2 = tp.tile([128, bc], F16, tag="t2")
            hh = tp.tile([128, bc], F16, tag="hh")
            dd = tp.tile([128, bc], F16, tag="dd")
            m1 = tp.tile([128, bc], F16, tag="m1")
            for gi in range(ng):
                ps_t = pzr.tile([128, 1024], F32, tag="ps")
                ps = ps_t[:]
                pG_t = pg.tile([128, 512], F32, tag="pg")
                pG = pG_t[:]
                exs = slice(gi * 512, (gi + 1) * 512)
                nc.tensor.matmul(ps[:, 0:512], ua_sb[:, 0:128], h[:, exs], start=True, stop=False)
                nc.tensor.matmul(ps[:, 0:512], wa_sb[:, 0:128], xgs[gi][0:XW, 0, :],
                                 start=False, stop=not with_czr)
                if with_czr:
                    nc.tensor.matmul(ps[:, 0:512], czr_sb[:, 0:128], onesbc_sb[:, exs],
                                     start=False, stop=True)
                nc.tensor.matmul(ps[:, 512:1024], ua_sb[:, 128:256], h[:, exs], start=True, stop=False)
                nc.tensor.matmul(ps[:, 512:1024], wa_sb[:, 128:256], xgs[gi][0:XW, 0, :],
                                 start=False, stop=not with_czr)
                if with_czr:
                    nc.tensor.matmul(ps[:, 512:1024], czr_sb[:, 128:256], onesbc_sb[:, exs],
                                     start=False, stop=True)
                nc.tensor.matmul(pG, ua_sb[:, 256:384], h[:, exs], start=True, stop=True)
                nc.scalar.activation(zr[:, gi * 1024:(gi + 1) * 1024], ps, AF.Sigmoid)
                nc.vector.scalar_tensor_tensor(
                    t1[:, exs], pG, b1h_sb[:],
                    zr[:, gi * 1024 + 512:(gi + 1) * 1024], OP.add, OP.mult,
                )
                nc.vector.tensor_add(t2[:, exs], t1[:, exs], xgs[gi][:, 1, :])

            def blend(pi):
                sl = slice(pi * pw, (pi + 1) * pw)
                nc.vector.tensor_sub(dd[:, sl], h[:, sl], hh[:, sl])
                zv = zr[:, pi * gperp * 1024:(pi + 1) * gperp * 1024] \
                    .rearrange("p (g c) -> p g c", g=gperp)[:, :, 0:512]
                dv = dd[:, sl].rearrange("p (g c) -> p g c", g=gperp)
                mv = m1[:, sl].rearrange("p (g c) -> p g c", g=gperp)
                nc.vector.tensor_mul(mv, zv, dv)
                nc.vector.tensor_add(hnew[:, sl], m1[:, sl], hh[:, sl])

            for pi in range(npairs):
                sl = slice(pi * pw, (pi + 1) * pw)
                nc.scalar.activation(hh[:, sl], t2[:, sl], AF.Tanh)
                blend(pi)
            h = hnew

        out_sb = hd.tile([128, (bc // 128) * C], F32, tag="out")
        et_all = hd.tile([128, (bc // 128) * C], F32, tag="eta")
        ss_all = hd.tile([128, (bc // 128)], F32, tag="ssa")
        for hg in range(bc // 512):
            psd_t = pzr.tile([128, 1024], F32, tag="ps")
            psd = psd_t[:, 0:512]
            nc.tensor.matmul(psd, w1_sb[:], h[:, hg * 512:(hg + 1) * 512], start=True, stop=True)
            sg = hd.tile([128, 512], F16, tag="sg")
            nc.scalar.activation(sg[:], psd, AF.Sigmoid, bias=b1c_sb[:])
            dt = hd.tile([128, 512], F16, tag="dt")
            # swish(d) = d * sigmoid(d), d = psd + b1
            nc.vector.scalar_tensor_tensor(dt[:], psd, b1c_sb[:], sg[:], OP.add, OP.mult)
            for sub in range(4):
                psl_t = pg.tile([128, 512], F32, tag="pg")
                psl = psl_t[:, 0:C]
                nc.tensor.matmul(psl, dt[:, sub * 128:(sub + 1) * 128], wout_sb[:], start=True, stop=False)
                nc.tensor.matmul(psl, ones_sb[:], bout_sb[:], start=False, stop=True)
                i = hg * 4 + sub
                nc.scalar.activation(et_all[:, i * C:(i + 1) * C], psl, AF.Exp,
                                     accum_out=ss_all[:, i:i + 1])
        rc_all = hd.tile([128, (bc // 128)], F32, tag="rc")
        nc.vector.reciprocal(rc_all[:], ss_all[:])
        for i in range(bc // 128):
            nc.vector.tensor_scalar_mul(out_sb[:, i * C:(i + 1) * C],
                                        et_all[:, i * C:(i + 1) * C], rc_all[:, i:i + 1])
        nc.sync.dma_start(outp, out_sb[:])

    nc.compile()
    return nc


def prep_tables(emb, W, U, b, W1, b1, Wout, bout):
    """Host-side weight preprocessing -> (shared input dict, with_czr flag)."""
    f16 = np.float16
    emb = np.asarray(emb, np.float64)
    W = np.asarray(W, np.float64)
    b = np.asarray(b, np.float64)
    xtab = np.zeros((V, RW), f16)
    xtab[:, 0:E] = emb.astype(f16)
    xtab[0, E] = np.float16(BIGM)        # mask indicator: token==0 freezes state
    xtab[:, E + 1] = 1.0                 # constant row (bias carrier for thin groups)
    # pre-projected h-gate input block (gtab trick, incl b0_h)
    xtab[:, H:2 * H] = (emb @ W[:, 2 * H:3 * H] + b[0, 2 * H:3 * H]).astype(f16)
    wa = np.zeros((XW, 2 * H), f16)
    wa[0:E, :] = W[:, 0:2 * H].astype(f16)
    wa[E, 0:H] = 1.0                     # routes the mask indicator into z-pre
    # constant z/r bias terms (b0+b1 both enter pre-sigmoid directly)
    c = b[0, 0:2 * H] + b[1, 0:2 * H]
    with_czr = bool(np.any(np.abs(c) > 1e-12))
    shared = {
        "xtab": xtab,
        "ua": np.asarray(U, np.float32).astype(f16),
        "wa": wa,
        "b1h": np.asarray(b[1, 2 * H:3 * H], np.float32).reshape(128, 1).copy(),
        "w1": np.asarray(W1, np.float32).astype(f16),
        "b1c": np.asarray(b1, np.float32).reshape(128, 1).copy(),
        "wout": np.asarray(Wout, np.float32).astype(f16),
        "boutw": np.asarray(bout, np.float32).reshape(1, C).astype(f16),
    }
    if with_czr:
        shared["czr"] = c.reshape(1, 256).astype(f16)
    return shared, with_czr


def prep_idx(tokens_core, nt):
    """tokens_core [bc, nt] int -> idx tensor [128, nt*bc/16] int16.

    Per step t, per call c (PERCALL idxs), position i = e - c*PERCALL is
    wrapped: [16, PERCALL/16] column-major, replicated to 128 partitions.
    dma_gather(transpose=False) then writes row(e) to out[p=e%128, e//128]."""
    bc = tokens_core.shape[0]
    tk = np.asarray(tokens_core, np.int16)
    w = tk.T.reshape(nt * bc // 16, 16).transpose(1, 0)   # [16, nt*bc/16]
    return np.ascontiguousarray(np.tile(w, (8, 1)))


def assemble_out(res_core, bc=BC):
    """[128, (bc/128)*3] f32 device output -> [bc, 3] (example e = i*128 + p)."""
    return np.ascontiguousarray(
        res_core.reshape(128, bc // 128, C).transpose(1, 0, 2).reshape(bc, C)
    )


_NC_CACHE = {}


def kernel(tokens, emb, W, U, b, W1, b1, Wout, bout):
    tokens = np.asarray(tokens)
    shared, with_czr = prep_tables(emb, W, U, b, W1, b1, Wout, bout)
    key = (BC, T, with_czr)
    if key not in _NC_CACHE:
        _NC_CACHE[key] = build_nc(BC, T, with_czr)
    nc = _NC_CACHE[key]
    in_maps = []
    for c in range(NCORES):
        m = dict(shared)
        tc = tokens[c * BC:(c + 1) * BC]
        m["idxw"] = prep_idx(tc, T)
        in_maps.append(m)
    res = run_bass_kernel_spmd(nc, in_maps, core_ids=list(range(NCORES)))
    out = np.concatenate([assemble_out(res.results[c]["outp"], BC) for c in range(NCORES)], axis=0)
    return out.astype(np.float32)


# revision 24
# speedup vs baseline: 1.4393x; 1.0011x over previous
"""Trainium2 Bass kernel for nn_AttentionRNN (embedding + masked GRU + MLP head + softmax).

Strategy (pure data parallelism over 8 NeuronCores, 2048 examples/core):

The GRU runs transposed — state h kept as hT [H=128 partitions, examples on
free dim], so the recurrence is closed under the layout.

Embedding lookup: 4 transpose-mode dma_gather calls per step (512 idxs each,
round-robin across 4 SWDGE queues so descriptor generation runs on 4 Q7 core
pairs in parallel, ~5us/step total) fetch 256B rows of xtab [V, 128] f16:
  xtab[v] = [emb[v] (32) | mask_ind (BIGM if v==0 else 0) | 0-pad (95)]
directly into feature-on-partition layout xaT [128, 2048] (partition f =
augmented-input feature f, token on free dim) — no on-chip transposes needed.
The mask indicator row of the augmented weight matrix wa adds BIGM to the
z-gate preactivation of masked steps => z = sigmoid(>=BIGM - |U@h|) == 1
=> h' = h exactly.

Each gathered 512B row = [x-aug block (256B) | pre-projected h-gate block
(256B)]: block 0 = [emb(32) | mask_ind | const 1 | pad] feeding the z/r
matmuls (K=34); block 1 = emb@W_h + b0_h (the gtab trick), consumed directly
by the DVE t2-add — no h-gate input matmul at all.

Per step t, per 512-example group (psum [128, 1024] = z|r + 1 bank pG):

  z-pre = U_z.T @ hT + Wa_z.T @ xaT   (Wa_z row 32 = 1s -> +BIGM when masked)
  r-pre = U_r.T @ hT + Wa_r.T @ xaT
  pG    = U_h.T @ hT                              (recurrent h-gate part)
  z|r  = sigmoid([z-pre | r-pre])                 (one ACT call, reads PSUM)
  t1   = (pG + b1_h) * r                          (DVE scalar_tensor_tensor)
  t2   = t1 + xghT                                (DVE add, gathered block 1)
  hh   = tanh(t2)                                 (ACT)
  h'   = z*(h - hh) + hh                          (3 DVE tensor_tensor ops)

Biases (zero for the graded input): b1_h via the STT scalar, b0_h baked into
the gathered block, b0_z|r + b1_z|r via K=1 matmuls (czr) only when nonzero.

Head: dT = swish(W1.T @ hT + b1); logits per 128-example tile with examples on
partitions (lhsT = dT slice); softmax along free dim (C=3).
"""

import numpy as np
from contextlib import ExitStack

import concourse.mybir as mybir
import concourse.tile as tile
from concourse import bacc
from concourse.bass_utils import run_bass_kernel_spmd

B, T, E, H, V, D, C = 16384, 128, 32, 128, 30001, 128, 3
NCORES = 8
BC = B // NCORES
BIGM = 100.0
RW = 256               # xtab row pitch in f16: [x-aug block | pre-projected h-gate block]
XW = 34                # useful columns of xtab (= rows of wa): E emb + mask + const
NQ = 4                 # SWDGE queues; gather calls round-robin across them
PERCALL = 512          # idxs per dma_gather call
DMA_SCRATCH = 49152    # SWDGE descriptor-ring carveout (bytes/partition)
QUEUE_PLAN = [0, 1, 2, 3]  # queue for call c
G_BUFS = 6
H_BUFS = 3
Z_BUFS = 3
TMP_BUFS = 2
XA_BUFS = 2
PW = 512               # width of the blend DVE ops
PZR_BUFS = 3           # psum pool depths (8 banks: 3*PZR + PG)
PG_BUFS = 2
PX_BUFS = 2
COPY_ENGINE = "act"    # psum->sbuf xaT copies: "act" or "dve"

F16 = mybir.dt.float16
F32 = mybir.dt.float32
I16 = mybir.dt.int16
AF = mybir.ActivationFunctionType
OP = mybir.AluOpType


def build_nc(bc=BC, nt=T, with_czr=False):
    """Build + compile the per-core Bass program. bc = examples per core."""
    assert bc % 512 == 0
    ng = bc // 512            # 512-example groups per step
    kb = bc // 128            # gathered rows per partition per step
    pw = min(PW, bc)
    npairs = bc // pw
    gperp = pw // 512

    nc = bacc.Bacc("TRN2", target_bir_lowering=False, debug=False,
                   num_swdge_queues=NQ, dynamic_dma_scratch_size=DMA_SCRATCH)
    xtab = nc.dram_tensor("xtab", [V, RW], F16, kind="ExternalInput").ap()
    idxw = nc.dram_tensor("idxw", [128, nt * bc // 16], I16, kind="ExternalInput").ap()
    ua = nc.dram_tensor("ua", [128, 384], F16, kind="ExternalInput").ap()
    wa = nc.dram_tensor("wa", [XW, 256], F16, kind="ExternalInput").ap()
    b1h = nc.dram_tensor("b1h", [128, 1], F32, kind="ExternalInput").ap()
    w1 = nc.dram_tensor("w1", [128, 128], F16, kind="ExternalInput").ap()
    b1c = nc.dram_tensor("b1c", [128, 1], F32, kind="ExternalInput").ap()
    wout = nc.dram_tensor("wout", [128, C], F16, kind="ExternalInput").ap()
    boutw = nc.dram_tensor("boutw", [1, C], F16, kind="ExternalInput").ap()
    if with_czr:
        czr = nc.dram_tensor("czr", [1, 256], F16, kind="ExternalInput").ap()
    outp = nc.dram_tensor("outp", [128, (bc // 128) * C], F32, kind="ExternalOutput").ap()

    with tile.TileContext(nc) as tc, ExitStack() as ctx:
        wp = ctx.enter_context(tc.tile_pool(name="w", bufs=1))
        ip = ctx.enter_context(tc.tile_pool(name="idx", bufs=1))
        gp = ctx.enter_context(tc.tile_pool(name="g", bufs=G_BUFS))
        hp = ctx.enter_context(tc.tile_pool(name="h", bufs=H_BUFS))
        zp = ctx.enter_context(tc.tile_pool(name="zr", bufs=Z_BUFS))
        tp = ctx.enter_context(tc.tile_pool(name="tmp", bufs=TMP_BUFS))
        pzr = ctx.enter_context(tc.tile_pool(name="pszr", bufs=PZR_BUFS, space="PSUM"))
        pg = ctx.enter_context(tc.tile_pool(name="psg", bufs=PG_BUFS, space="PSUM"))
        hd = ctx.enter_context(tc.tile_pool(name="hd", bufs=2))

        ua_sb = wp.tile([128, 384], F16, tag="ua")
        nc.sync.dma_start(ua_sb[:], ua)
        wa_sb = wp.tile([XW, 256], F16, tag="wa")
        nc.sync.dma_start(wa_sb[:], wa)
        b1h_sb = wp.tile([128, 1], F32, tag="b1h")
        nc.sync.dma_start(b1h_sb[:], b1h)
        w1_sb = wp.tile([128, 128], F16, tag="w1")
        nc.sync.dma_start(w1_sb[:], w1)
        b1c_sb = wp.tile([128, 1], F32, tag="b1c")
        nc.sync.dma_start(b1c_sb[:], b1c)
        wout_sb = wp.tile([128, C], F16, tag="wo")
        nc.sync.dma_start(wout_sb[:], wout)
        bout_sb = wp.tile([1, C], F16, tag="bo")
        nc.sync.dma_start(bout_sb[:], boutw)
        ones_sb = wp.tile([1, 128], F16, tag="ones")
        nc.vector.memset(ones_sb[:], 1.0)
        # Pin the ACT table set that contains BOTH Sigmoid and Tanh so the
        # auto-placement pass doesn't ping-pong table loads every step
        # (~1.3us per load on the ACT critical path).
        from concourse.hw_specs import get_activation_tables
        _tabs = get_activation_tables(nc.m.arch)
        _setid = next(i for i, (nm2, fs) in enumerate(_tabs.items())
                      if AF.Sigmoid in fs and AF.Tanh in fs)
        nc.scalar.add_instruction(mybir.InstLoadActFuncSet(
            name=nc.get_next_instruction_name(), ins=[], outs=[],
            act_func_set_id=_setid))
        if with_czr:
            czr_sb = wp.tile([1, 256], F16, tag="czr")
            nc.sync.dma_start(czr_sb[:], czr)
            onesbc_sb = wp.tile([1, bc], F16, tag="onesbc")
            nc.vector.memset(onesbc_sb[:], 1.0)
        idx_sb = ip.tile([128, nt * bc // 16], I16, tag="idx")
        nc.sync.dma_start(idx_sb[:], idxw)

        h = hp.tile([128, bc], F16, tag="h")
        nc.vector.memset(h[:], 0.0)

        for t in range(nt):
            ncall = bc // PERCALL
            xgs = []
            for c in range(ncall):
                xgc = gp.tile([128, 2, PERCALL], F16, tag="g")
                off = t * bc + c * PERCALL
                nc.gpsimd.dma_gather(
                    xgc[:], xtab,
                    idx_sb[:, off // 16:(off + PERCALL) // 16],
                    PERCALL, PERCALL, RW, transpose=True,
                    queue_num=QUEUE_PLAN[c % len(QUEUE_PLAN)],
                )
                xgs.append(xgc)

            zr = zp.tile([128, 2 * bc], F16, tag="zr")
            t1 = tp.tile([128, bc], F16, tag="t1")
            hnew = hp.tile([128, bc], F16, tag="h")
            t2 = tp.tile([128, bc], F16, tag="t2")
            hh = tp.tile([128, bc], F16, tag="hh")
            dd = tp.tile([128, bc], F16, tag="dd")
            m1 = tp.tile([128, bc], F16, tag="m1")
            for gi in range(ng):
                ps_t = pzr.tile([128, 1024], F32, tag="ps")
                ps = ps_t[:]
                pG_t = pg.tile([128, 512], F32, tag="pg")
                pG = pG_t[:]
                exs = slice(gi * 512, (gi + 1) * 512)
                nc.tensor.matmul(ps[:, 0:512], ua_sb[:, 0:128], h[:, exs], start=True, stop=False)
                nc.tensor.matmul(ps[:, 0:512], wa_sb[:, 0:128], xgs[gi][0:XW, 0, :],
                                 start=False, stop=not with_czr)
                if with_czr:
                    nc.tensor.matmul(ps[:, 0:512], czr_sb[:, 0:128], onesbc_sb[:, exs],
                                     start=False, stop=True)
                nc.tensor.matmul(ps[:, 512:1024], ua_sb[:, 128:256], h[:, exs], start=True, stop=False)
                nc.tensor.matmul(ps[:, 512:1024], wa_sb[:, 128:256], xgs[gi][0:XW, 0, :],
                                 start=False, stop=not with_czr)
                if with_czr:
                    nc.tensor.matmul(ps[:, 512:1024], czr_sb[:, 128:256], onesbc_sb[:, exs],
                                     start=False, stop=True)
                nc.tensor.matmul(pG, ua_sb[:, 256:384], h[:, exs], start=True, stop=True)
                nc.scalar.activation(zr[:, gi * 1024:(gi + 1) * 1024], ps, AF.Sigmoid)
                nc.vector.scalar_tensor_tensor(
                    t1[:, exs], pG, b1h_sb[:],
                    zr[:, gi * 1024 + 512:(gi + 1) * 1024], OP.add, OP.mult,
                )
                nc.vector.tensor_add(t2[:, exs], t1[:, exs], xgs[gi][:, 1, :])

            def blend(pi):
                sl = slice(pi * pw, (pi + 1) * pw)
                nc.vector.tensor_sub(dd[:, sl], h[:, sl], hh[:, sl])
                zv = zr[:, pi * gperp * 1024:(pi + 1) * gperp * 1024] \
                    .rearrange("p (g c) -> p g c", g=gperp)[:, :, 0:512]
                dv = dd[:, sl].rearrange("p (g c) -> p g c", g=gperp)
                mv = m1[:, sl].rearrange("p (g c) -> p g c", g=gperp)
                nc.vector.tensor_mul(mv, zv, dv)
                nc.vector.tensor_add(hnew[:, sl], m1[:, sl], hh[:, sl])

            for pi in range(npairs):
                sl = slice(pi * pw, (pi + 1) * pw)
                nc.scalar.activation(hh[:, sl], t2[:, sl], AF.Tanh)
                blend(pi)
            h = hnew

        out_sb = hd.tile([128, (bc // 128) * C], F32, tag="out")
        et_all = hd.tile([128, (bc // 128) * C], F32, tag="eta")
        ss_all = hd.tile([128, (bc // 128)], F32, tag="ssa")
        for hg in range(bc // 512):
            psd_t = pzr.tile([128, 1024], F32, tag="ps")
            psd = psd_t[:, 0:512]
            nc.tensor.matmul(psd, w1_sb[:], h[:, hg * 512:(hg + 1) * 512], start=True, stop=True)
            sg = hd.tile([128, 512], F16, tag="sg")
            nc.scalar.activation(sg[:], psd, AF.Sigmoid, bias=b1c_sb[:])
            dt = hd.tile([128, 512], F16, tag="dt")
            # swish(d) = d * sigmoid(d), d = psd + b1
            nc.vector.scalar_tensor_tensor(dt[:], psd, b1c_sb[:], sg[:], OP.add, OP.mult)
            for sub in range(4):
                psl_t = pg.tile([128, 512], F32, tag="pg")
                psl = psl_t[:, 0:C]
                nc.tensor.matmul(psl, dt[:, sub * 128:(sub + 1) * 128], wout_sb[:], start=True, stop=False)
                nc.tensor.matmul(psl, ones_sb[:], bout_sb[:], start=False, stop=True)
                i = hg * 4 + sub
                nc.scalar.activation(et_all[:, i * C:(i + 1) * C], psl, AF.Exp,
                                     accum_out=ss_all[:, i:i + 1])
        rc_all = hd.tile([128, (bc // 128)], F32, tag="rc")
        nc.vector.reciprocal(rc_all[:], ss_all[:])
        for i in range(bc // 128):
            nc.vector.tensor_scalar_mul(out_sb[:, i * C:(i + 1) * C],
                                        et_all[:, i * C:(i + 1) * C], rc_all[:, i:i + 1])
        nc.sync.dma_start(outp, out_sb[:])

    nc.compile()
    return nc


def prep_tables(emb, W, U, b, W1, b1, Wout, bout):
    """Host-side weight preprocessing -> (shared input dict, with_czr flag)."""
    f16 = np.float16
    emb = np.asarray(emb, np.float64)
    W = np.asarray(W, np.float64)
    b = np.asarray(b, np.float64)
    xtab = np.zeros((V, RW), f16)
    xtab[:, 0:E] = emb.astype(f16)
    xtab[0, E] = np.float16(BIGM)        # mask indicator: token==0 freezes state
    xtab[:, E + 1] = 1.0                 # constant row (bias carrier for thin groups)
    # pre-projected h-gate input block (gtab trick, incl b0_h)
    xtab[:, H:2 * H] = (emb @ W[:, 2 * H:3 * H] + b[0, 2 * H:3 * H]).astype(f16)
    wa = np.zeros((XW, 2 * H), f16)
    wa[0:E, :] = W[:, 0:2 * H].astype(f16)
    wa[E, 0:H] = 1.0                     # routes the mask indicator into z-pre
    # constant z/r bias terms (b0+b1 both enter pre-sigmoid directly)
    c = b[0, 0:2 * H] + b[1, 0:2 * H]
    with_czr = bool(np.any(np.abs(c) > 1e-12))
    shared = {
        "xtab": xtab,
        "ua": np.asarray(U, np.float32).astype(f16),
        "wa": wa,
        "b1h": np.asarray(b[1, 2 * H:3 * H], np.float32).reshape(128, 1).copy(),
        "w1": np.asarray(W1, np.float32).astype(f16),
        "b1c": np.asarray(b1, np.float32).reshape(128, 1).copy(),
        "wout": np.asarray(Wout, np.float32).astype(f16),
        "boutw": np.asarray(bout, np.float32).reshape(1, C).astype(f16),
    }
    if with_czr:
        shared["czr"] = c.reshape(1, 256).astype(f16)
    return shared, with_czr


def prep_idx(tokens_core, nt):
    """tokens_core [bc, nt] int -> idx tensor [128, nt*bc/16] int16.

    Per step t, per call c (PERCALL idxs), position i = e - c*PERCALL is
    wrapped: [16, PERCALL/16] column-major, replicated to 128 partitions.
    dma_gather(transpose=False) then writes row(e) to out[p=e%128, e//128]."""
    bc = tokens_core.shape[0]
    tk = np.asarray(tokens_core, np.int16)
    w = tk.T.reshape(nt * bc // 16, 16).transpose(1, 0)   # [16, nt*bc/16]
    return np.ascontiguousarray(np.tile(w, (8, 1)))


def assemble_out(res_core, bc=BC):
    """[128, (bc/128)*3] f32 device output -> [bc, 3] (example e = i*128 + p)."""
    return np.ascontiguousarray(
        res_core.reshape(128, bc // 128, C).transpose(1, 0, 2).reshape(bc, C)
    )


_NC_CACHE = {}


def kernel(tokens, emb, W, U, b, W1, b1, Wout, bout):
    tokens = np.asarray(tokens)
    shared, with_czr = prep_tables(emb, W, U, b, W1, b1, Wout, bout)
    key = (BC, T, with_czr)
    if key not in _NC_CACHE:
        _NC_CACHE[key] = build_nc(BC, T, with_czr)
    nc = _NC_CACHE[key]
    in_maps = []
    for c in range(NCORES):
        m = dict(shared)
        tc = tokens[c * BC:(c + 1) * BC]
        m["idxw"] = prep_idx(tc, T)
        in_maps.append(m)
    res = run_bass_kernel_spmd(nc, in_maps, core_ids=list(range(NCORES)))
    out = np.concatenate([assemble_out(res.results[c]["outp"], BC) for c in range(NCORES)], axis=0)
    return out.astype(np.float32)
